# revision 4
# baseline (speedup 1.0000x reference)
"""Trainium2 Bass kernel for cosine-similarity multi-head attention.

Math (per batch element b):
    context = query @ w_q.T + b_q                    # [S, 120]
    ctx     = context * weight_tensor                # bcast [1,120]
    ctx_n   = ctx / max(||ctx||_2(axis=-1), 1e-12)   # L2 normalize
    scores  = ctx_n @ ctx_n.T                        # [S, S]
    out     = softmax(where(mask==0, -1e9, scores))  # row softmax
Sharding: data-parallel over batch. 8 batch elements -> 8 NeuronCores.

Phase 1 computes the transposed normalized context ctxT [120, S]
directly on the PE array (no DMA-XBAR transposes -- the XBAR path
raced with concurrent SBUF traffic and corrupted tokens):
  per 128-token tile: query arrives bf16 via casting SWDGE DMA, is
  PE-transposed (identity matmul) to qT, and the projection matmul is
  run "flipped" (lhsT = (w_q*wt).T chunks, rhs = qT chunks) so the
  output PSUM tile is already [feature, token].  Norms are per-column
  (= per token), which the PE reduces for free: ones[120,1].T @
  ctx^2 -> [1, tokens].  rstd = Rsqrt (ACT), partition_broadcast
  (gpsimd), one DVE multiply -> normalized bf16 ctxT.
Phase 2: per 128-row q-tile: PE matmul scores chunks (bf16) -> ACT exp
  -> DVE tensor_tensor_reduce (mask multiply + row-sum fused, in place
  over the mask tile) -> reciprocal -> scaled bf16 copy -> DMA out.
  Softmax skips the row-max subtraction: scores are cosine
  similarities in [-1, 1], and masked entries are exactly zeroed by
  the mask multiply.  Output is bf16 (~0.4% rounding, far inside the
  2e-2 tolerance); host upcasts to fp32.
"""

import sys

if "/opt/trn_rl_repo" not in sys.path:
    sys.path.insert(0, "/opt/trn_rl_repo")

from contextlib import ExitStack

import numpy as np

import concourse.bass as bass
import concourse.mybir as mybir
import concourse.tile as tile
from concourse import bacc
from concourse.dve_ops import TENSOR_TENSOR_REDUCE as TTR_OP
from concourse.masks import make_identity

D_MODEL = 512
H_DIM = 120
N_CORES = 8
P = 128  # partition tile

F32 = mybir.dt.float32
BF16 = mybir.dt.bfloat16
I32 = mybir.dt.int32
Alu = mybir.AluOpType
Act = mybir.ActivationFunctionType

CFG = dict(
    chunk=2048,      # phase-2 column chunk (multiple of 512)
    mask_bufs=6,     # int32 mask tiles; tile doubles as softmax scratch
    ech_bufs=3,      # fp32 exp-chunk temps [128, chunk]
    obuf_bufs=3,     # bf16 store tiles [128, S]
    ps2_bufs=2,      # phase-2 psum tiles [128, chunk] (4 banks each)
    ngrp=4,          # phase-1 norm batch (tiles per Rsqrt batch)
)


def build_nc(S: int = 4096):
    nc = bacc.Bacc("TRN2", target_bir_lowering=False, debug=False)

    q_dram = nc.dram_tensor("query", [S, D_MODEL], F32, kind="ExternalInput")
    m_dram = nc.dram_tensor("mask", [S, S], I32, kind="ExternalInput")
    wq_dram = nc.dram_tensor("w_q", [H_DIM, D_MODEL], F32, kind="ExternalInput")
    bq_dram = nc.dram_tensor("b_q", [H_DIM], F32, kind="ExternalInput")
    wt_dram = nc.dram_tensor("weight_tensor", [1, H_DIM], F32, kind="ExternalInput")
    out_dram = nc.dram_tensor("out", [S, S], BF16, kind="ExternalOutput")

    NT = S // P                      # 128-row tiles
    CHUNK = min(CFG["chunk"], S)
    NCH = S // CHUNK
    ND = D_MODEL // P                # 4 chunks of contraction dim
    G = CFG["ngrp"]
    NG = NT // G

    with tile.TileContext(nc) as tc, ExitStack() as ctx:
        singles = ctx.enter_context(tc.tile_pool(name="singles", bufs=1))

        # ---------- Phase 0: constants ----------
        ident = singles.tile([P, P], F32)
        make_identity(nc, ident)
        ident_bf = singles.tile([P, P], BF16)
        nc.scalar.copy(ident_bf, ident)

        # weight_tensor broadcast to all 128 partitions: [128, 120]
        wtb = singles.tile([P, H_DIM], F32)
        nc.gpsimd.dma_start(
            out=wtb,
            in_=bass.AP(tensor=wt_dram, offset=0, ap=[[0, P], [1, H_DIM]]),
        )

        # b_q * weight_tensor -> bw [1, 120] (bf16 for the bias matmul)
        bq_sb = singles.tile([1, H_DIM], F32)
        nc.gpsimd.dma_start(
            out=bq_sb,
            in_=bass.AP(tensor=bq_dram, offset=0, ap=[[0, 1], [1, H_DIM]]),
        )
        bw = singles.tile([1, H_DIM], F32)
        nc.vector.tensor_mul(bw, bq_sb, wtb[:1, :])
        bw_bf = singles.tile([1, H_DIM], BF16)
        nc.scalar.copy(bw_bf, bw)

        ones_row = singles.tile([1, P], BF16)
        nc.vector.memset(ones_row, 1.0)
        ones_col = singles.tile([H_DIM, 1], BF16)
        nc.vector.memset(ones_col, 1.0)

        # w_q [120, 512] -> transposed+scaled bf16 wqTs [4x128, 120]
        wq_sb = singles.tile([H_DIM, D_MODEL], F32)
        nc.sync.dma_start(out=wq_sb, in_=wq_dram.ap())
        wqTs = singles.tile([P, ND * H_DIM], BF16)

        # persistent normalized-transposed context, bf16 [120 (pad 128), S]
        ctxT = singles.tile([P, S], BF16)
        # raw (unnormalized) bf16 context, same layout
        ctxU = singles.tile([P, S], BF16)

        with ExitStack() as ph0:
            ps_w = ph0.enter_context(
                tc.tile_pool(name="ps_w", bufs=2, space="PSUM"))
            wq_f = singles.tile([P, ND * H_DIM], F32)
            for c in range(ND):
                wqT_ps = ps_w.tile([P, H_DIM], F32)
                nc.tensor.transpose(
                    wqT_ps, wq_sb[:, c * P:(c + 1) * P], ident[:H_DIM, :H_DIM])
                # evict + fold in weight_tensor scale
                nc.vector.tensor_mul(
                    wq_f[:, c * H_DIM:(c + 1) * H_DIM], wqT_ps, wtb)
            nc.scalar.copy(wqTs, wq_f)

        # phase-2 SBUF pools created before phase-1 scratch so the deep
        # mask prefetch never aliases phase-1 buffers.
        mask_p = ctx.enter_context(
            tc.tile_pool(name="maskp", bufs=CFG["mask_bufs"]))
        ech_p = ctx.enter_context(tc.tile_pool(name="echp", bufs=CFG["ech_bufs"]))
        obuf_p = ctx.enter_context(
            tc.tile_pool(name="obufp", bufs=CFG["obuf_bufs"]))
        sum_p = ctx.enter_context(tc.tile_pool(name="sump", bufs=3))

        with ExitStack() as ph1:
            # ---------- Phase 1: build ctxT (PE transposes only) ----------
            qin_p = ph1.enter_context(tc.tile_pool(name="qin", bufs=3))
            qt_p = ph1.enter_context(tc.tile_pool(name="qt", bufs=3))
            sq_p = ph1.enter_context(tc.tile_pool(name="sq", bufs=2))
            nrow_p = ph1.enter_context(tc.tile_pool(name="nrow", bufs=2))
            rrow_p = ph1.enter_context(tc.tile_pool(name="rrow", bufs=2))
            rb_p = ph1.enter_context(tc.tile_pool(name="rb", bufs=2))
            ps_q = ph1.enter_context(
                tc.tile_pool(name="ps_q", bufs=2, space="PSUM"))
            ps_c = ph1.enter_context(
                tc.tile_pool(name="ps_c", bufs=2, space="PSUM"))
            ps_n = ph1.enter_context(
                tc.tile_pool(name="ps_n", bufs=2, space="PSUM"))

            for g in range(NG):
                n2_ps = ps_n.tile([1, G * P], F32)
                for ii in range(G):
                    i = g * G + ii
                    s0 = i * P
                    # query arrives bf16 via casting SWDGE DMA
                    q_bf = qin_p.tile([P, D_MODEL], BF16)
                    nc.gpsimd.dma_start(out=q_bf, in_=q_dram[s0:s0 + P, :])

                    # PE-transpose the four 128x128 blocks -> qT [d, s]
                    qT_ps = ps_q.tile([P, D_MODEL], BF16)
                    for c in range(ND):
                        nc.tensor.transpose(
                            qT_ps[:, c * P:(c + 1) * P],
                            q_bf[:, c * P:(c + 1) * P], ident_bf)
                    qT_sb = qt_p.tile([P, D_MODEL], BF16)
                    nc.vector.tensor_copy(qT_sb, qT_ps)

                    # flipped projection: ctx_ps [feature=120, token=128]
                    ctx_ps = ps_c.tile([H_DIM, P], F32)
                    for c in range(ND):
                        nc.tensor.matmul(
                            ctx_ps,
                            lhsT=wqTs[:, c * H_DIM:(c + 1) * H_DIM],
                            rhs=qT_sb[:, c * P:(c + 1) * P],
                            start=(c == 0), stop=False)
                    nc.tensor.matmul(
                        ctx_ps, lhsT=bw_bf, rhs=ones_row,
                        start=False, stop=True)

                    # squares (ACT) + raw bf16 eviction (DVE)
                    sq_bf = sq_p.tile([H_DIM, P], BF16)
                    nc.scalar.activation(sq_bf, ctx_ps, Act.Square)
                    nc.vector.tensor_copy(ctxU[:H_DIM, s0:s0 + P], ctx_ps)
                    # per-token norm^2 via PE partition-reduce
                    nc.tensor.matmul(
                        n2_ps[:, ii * P:(ii + 1) * P],
                        lhsT=ones_col, rhs=sq_bf, start=True, stop=True)

                # batched normalize for the group's G*128 tokens
                c0 = g * G * P
                c1 = (g + 1) * G * P
                n2row = nrow_p.tile([1, G * P], F32)
                nc.vector.tensor_copy(n2row, n2_ps)
                # rstd = 2/(s0 + x/s0)  (sqrt + one Newton step, inverted)
                sroot = rrow_p.tile([1, G * P], F32, tag="sroot")
                nc.scalar.activation(sroot, n2row, Act.Sqrt)
                r0 = rrow_p.tile([1, G * P], F32, tag="r0")
                nc.vector.reciprocal(r0, sroot)
                t1 = rrow_p.tile([1, G * P], F32, tag="t1")
                nc.gpsimd.tensor_mul(t1, n2row, r0)
                ssum = rrow_p.tile([1, G * P], F32, tag="ssum")
                nc.gpsimd.tensor_add(ssum, sroot, t1)
                nc.gpsimd.tensor_scalar_max(ssum, ssum, 2e-12)
                u = rrow_p.tile([1, G * P], F32, tag="u")
                nc.vector.reciprocal(u, ssum)
                rrow = rrow_p.tile([1, G * P], F32, tag="rrow")
                nc.gpsimd.tensor_scalar_mul(rrow, u, 2.0)
                rB = rb_p.tile([P, G * P], F32)
                nc.gpsimd.partition_broadcast(rB, rrow)
                nc.vector.tensor_mul(
                    ctxT[:H_DIM, c0:c1], ctxU[:H_DIM, c0:c1], rB[:H_DIM, :])

        # ---------- Phase 2: scores + masked softmax ----------
        with ExitStack() as ph2:
            ps2 = ph2.enter_context(
                tc.tile_pool(name="ps2", bufs=CFG["ps2_bufs"], space="PSUM"))

            for i in range(NT):
                q0 = i * P
                mask_sb = mask_p.tile([P, S], I32)
                nc.sync.dma_start(out=mask_sb, in_=m_dram[q0:q0 + P, :])
                # f32 view of the same bytes: masked exp overwrites the
                # mask tile in place
                maskf = mask_sb.bitcast(F32)

                obuf = obuf_p.tile([P, S], BF16)
                sums = sum_p.tile([P, NCH], F32, tag="sums")
                lhsT = ctxT[:H_DIM, q0:q0 + P]
                for j in range(NCH):
                    c0 = j * CHUNK
                    sc_ps = ps2.tile([P, CHUNK], F32)
                    for h in range(CHUNK // 512):
                        nc.tensor.matmul(
                            sc_ps[:, h * 512:(h + 1) * 512],
                            lhsT=lhsT,
                            rhs=ctxT[:H_DIM, c0 + h * 512:c0 + (h + 1) * 512],
                            start=True, stop=True)
                    # exp (scores in [-1, 1]; masked entries zeroed next)
                    ech = ech_p.tile([P, CHUNK], F32)
                    nc.scalar.activation(ech, sc_ps, Act.Exp)
                    # fused mask-multiply + row-sum (chained across chunks);
                    # custom-DVE uop: out = in0*in1*s1, accum = s0 + sum(out)
                    nc.vector._custom_dve(
                        TTR_OP,
                        out=maskf[:, c0:c0 + CHUNK],
                        in0=ech,
                        in1=mask_sb[:, c0:c0 + CHUNK],
                        s0=(0.0 if j == 0 else sums[:, j - 1:j]),
                        s1=1.0,
                        accum_out=sums[:, j:j + 1])

                rden = sum_p.tile([P, 1], F32, tag="rden")
                nc.vector.reciprocal(rden, sums[:, NCH - 1:NCH])

                # normalize + fp32->bf16 in one ACT pass per chunk, then
                # store the half-width bf16 buffer
                for j in range(NCH):
                    c0 = j * CHUNK
                    nc.scalar.activation(
                        obuf[:, c0:c0 + CHUNK], maskf[:, c0:c0 + CHUNK],
                        Act.Copy, scale=rden)
                nc.scalar.dma_start(
                    out=out_dram[q0:q0 + P, :], in_=obuf)

    nc.compile()
    return nc


def _run(nc, in_maps, trace=False, tmpdir=None):
    from concourse import bass_utils
    return bass_utils.run_bass_kernel_spmd(
        nc, in_maps, core_ids=list(range(len(in_maps))), trace=trace,
        tmpdir=tmpdir)


def kernel(**inputs: np.ndarray) -> np.ndarray:
    query = np.ascontiguousarray(np.asarray(inputs["query"], np.float32))
    mask = np.ascontiguousarray(np.asarray(inputs["mask"], np.int32))
    w_q = np.ascontiguousarray(np.asarray(inputs["w_q"], np.float32))
    b_q = np.ascontiguousarray(np.asarray(inputs["b_q"], np.float32))
    wt = np.ascontiguousarray(
        np.asarray(inputs["weight_tensor"], np.float32).reshape(1, H_DIM))

    B, S, _ = query.shape
    assert B == N_CORES
    nc = build_nc(S)
    in_maps = [
        dict(query=query[b], mask=mask[b], w_q=w_q, b_q=b_q, weight_tensor=wt)
        for b in range(B)
    ]
    res = _run(nc, in_maps)
    return np.stack(
        [np.asarray(res.results[b]["out"]).astype(np.float32)
         for b in range(B)], axis=0)
